# revision 1
# baseline (speedup 1.0000x reference)
"""Trainium2 Bass kernel for a dense transformer block (attention + LoRA +
MLP + proj), data-parallel over batch across 8 NeuronCores.

Contract: kernel(**inputs) takes the FULL unsharded inputs (numpy arrays,
keys as in reference.setup_inputs()) and returns the FULL [8, 512, 1024]
fp32 output.

Design (per core, one batch element):
  - Everything flows channel-major ("transposed"): activations are [C, S]
    tiles with channels on SBUF partitions.  All weights are used in their
    natural [C_in, C_out] layout; the only input/output transposes happen
    on the host.
  - Attention runs keys-on-partitions (attnT = K q^T per head).  The key
    mask is folded into v (masked key rows of token-major v and of its
    appended ones-columns are zeroed), so softmax exp is a bias-free ACT
    op with the 1/sqrt(hd) scale folded in, and the denominator comes free
    as a ones-column in the PV matmul (M=65).  Heads are software-
    pipelined: head h's QK matmuls interleave 1:1 with head h-1's PV
    matmuls (the PE executes its stream in order, so PV - which waits on
    exp - must not block the next head's QK; the interleave also avoids
    back-to-back accumulation into one PSUM bank, which halves matmul
    rate).
  - PSUM: 2-bank "qk2" tiles (x3) released right after exp, 1-bank "pv"
    tiles (x2) that also serve the LoRA-tT and normalization matmuls.
  - Softmax normalization: per-head denominators are scattered to a
    [128, H, 4] layout so the DVE reciprocal runs on all 128 partitions
    (its cost is per-partition-serial), then PE transposes move the
    reciprocals to a [16, 512] queries-on-free layout, and a K=16
    selection matmul broadcasts them per chunk.
  - GEMMs run in bf16 (measured ~2x faster than fp32r); PSUM accumulation
    is fp32; the reciprocal path stays f32r (= fp32 bits).
"""

import numpy as np

B, S, C = 8, 512, 1024
H, HD, R, HID = 16, 64, 32, 1024
NC3 = 3 * C
NCORES = 8
KC = C // 128          # 8 contraction chunks
MQK = 2 * C // 128     # 16 q+k channel-major output chunks
VSTRIDE = HD + 1       # v columns per head incl. ones column

_cache = {}


def _get_nc():
    if "nc" in _cache:
        return _cache["nc"]

    from contextlib import ExitStack
    import concourse.tile as tile
    from concourse import bacc, mybir

    f32 = mybir.dt.float32
    f32r = mybir.dt.float32r
    bf16 = mybir.dt.bfloat16
    AF = mybir.ActivationFunctionType
    ALU = mybir.AluOpType

    nc = bacc.Bacc("TRN2", target_bir_lowering=False, debug=False)

    def din(name, shape, dt=bf16):
        return nc.dram_tensor(name, list(shape), dt, kind="ExternalInput")

    xT_d = din("xT", (C, S))
    mask01_d = din("mask01", (128, 4), f32)
    sel8_d = din("sel8", (8, 512), f32r)
    ident_d = din("ident", (128, 128), f32r)
    qkv_w_d = din("qkv_w", (C, NC3))
    qkv_la_d = din("qkv_la", (C, R))
    qkv_lb_d = din("qkv_lb", (R, NC3))
    proj_w_d = din("proj_w", (C, C))
    proj_b_d = din("proj_b", (C,), f32)
    proj_la_d = din("proj_la", (C, R))
    proj_lb_d = din("proj_lb", (R, C))
    fc1_w_d = din("fc1_w", (C, HID))
    fc1_b_d = din("fc1_b", (HID,), f32)
    fc1_la_d = din("fc1_la", (C, R))
    fc1_lb_d = din("fc1_lb", (R, HID))
    fc2_w_d = din("fc2_w", (HID, C))
    fc2_b_d = din("fc2_b", (C,), f32)
    fc2_la_d = din("fc2_la", (HID, R))
    fc2_lb_d = din("fc2_lb", (R, C))
    outT_d = nc.dram_tensor("outT", [C, S], f32, kind="ExternalOutput")

    with tile.TileContext(nc) as tc, ExitStack() as ctx:
        resident = ctx.enter_context(tc.tile_pool(name="resident", bufs=1))
        wpool = ctx.enter_context(tc.tile_pool(name="wstream", bufs=10))
        psum = ctx.enter_context(tc.tile_pool(name="psum", bufs=3, space="PSUM"))
        psum1 = ctx.enter_context(
            tc.tile_pool(name="psum1", bufs=2, space="PSUM")
        )
        expp = ctx.enter_context(tc.tile_pool(name="expp", bufs=2))
        tmpp = ctx.enter_context(tc.tile_pool(name="tmpp", bufs=2))
        outp = ctx.enter_context(tc.tile_pool(name="outp", bufs=2))

        def qk2_psum(name, dt=f32):
            # 2 PSUM banks; 3 bufs -> 6 banks
            return psum.tile([128, 2, S], dt, name=name, tag="qk2")

        def pv_psum(name, dt=f32):
            # 1 PSUM bank; 2 bufs -> 2 banks
            return psum1.tile([128, S], dt, name=name, tag="pv")

        # ---- resident loads -------------------------------------------------
        xT = resident.tile([128, KC, S], bf16, name="xT", tag="xT")
        xT_r = xT_d[:].rearrange("(c p) s -> p c s", p=128)
        for kc in range(KC):
            nc.gpsimd.dma_start(xT[:, kc, :], xT_r[:, kc, :])
        mask01 = resident.tile([128, 4], f32, name="mask01", tag="mask01")
        nc.gpsimd.dma_start(mask01[:], mask01_d[:])
        sel8 = resident.tile([8, 512], f32r, name="sel8", tag="sel8")
        nc.gpsimd.dma_start(sel8[:], sel8_d[:])
        ident = resident.tile([128, 128], f32r, name="ident", tag="ident")
        nc.gpsimd.dma_start(ident[:], ident_d[:])

        la = {}
        lb = {}
        for nm, la_d, lb_d, ncols in (
            ("qkv", qkv_la_d, qkv_lb_d, NC3),
            ("fc1", fc1_la_d, fc1_lb_d, HID),
            ("fc2", fc2_la_d, fc2_lb_d, C),
            ("proj", proj_la_d, proj_lb_d, C),
        ):
            la[nm] = resident.tile(
                [128, KC, R], bf16, name=f"la_{nm}", tag=f"la_{nm}"
            )
            nc.gpsimd.dma_start(
                la[nm][:], la_d[:].rearrange("(c p) r -> p c r", p=128)
            )
            lb[nm] = resident.tile(
                [R, ncols], bf16, name=f"lb_{nm}", tag=f"lb_{nm}"
            )
            nc.gpsimd.dma_start(lb[nm][:], lb_d[:])

        biases = {}
        for nm, b_d in (("fc1", fc1_b_d), ("fc2", fc2_b_d), ("proj", proj_b_d)):
            biases[nm] = resident.tile(
                [128, KC], f32, name=f"b_{nm}", tag=f"b_{nm}"
            )
            nc.gpsimd.dma_start(
                biases[nm][:], b_d[:].rearrange("(m p) -> p m", p=128)
            )

        qkv_w_r = qkv_w_d[:].rearrange("(k p) n -> k p n", p=128)
        fc1_w_r = fc1_w_d[:].rearrange("(k p) n -> k p n", p=128)
        fc2_w_r = fc2_w_d[:].rearrange("(k p) n -> k p n", p=128)
        proj_w_r = proj_w_d[:].rearrange("(k p) n -> k p n", p=128)

        def lora_step(nm, pt, act, kc):
            nc.tensor.matmul(
                pt[0:R, :], la[nm][:, kc, :], act[:, kc, :],
                start=(kc == 0), stop=(kc == KC - 1),
            )

        def lora_end(nm, pt):
            t = resident.tile([R, S], bf16, name=f"tT_{nm}", tag=f"tT_{nm}")
            nc.any.tensor_copy(t[:], pt[0:R, :])
            return t

        def mlp_gemm(nm, w_r, act, epilogue):
            """Generic 1024->1024 GEMM with LoRA; epilogue(m, psum_ap)."""
            pt = pv_psum(f"pt_{nm}")
            tT = None
            for g in range(2):
                pga = qk2_psum(f"p{nm}{g}a")
                pgb = qk2_psum(f"p{nm}{g}b")
                halves = (pga, pgb)
                for kc in range(KC):
                    wt = wpool.tile([128, 512], bf16, tag="w")
                    nc.sync.dma_start(
                        wt[:], w_r[kc, :, g * 512:(g + 1) * 512]
                    )
                    for i in range(4):
                        nc.tensor.matmul(
                            halves[i // 2][:, i % 2, :],
                            wt[:, i * 128:(i + 1) * 128],
                            act[:, kc, :], start=(kc == 0), stop=False,
                        )
                    if g == 0:
                        lora_step(nm, pt, act, kc)
                if g == 0:
                    tT = lora_end(nm, pt)
                for i in range(4):
                    m = g * 4 + i
                    pm = halves[i // 2][:, i % 2, :]
                    nc.tensor.matmul(
                        pm, lb[nm][:, m * 128:(m + 1) * 128],
                        tT[:], start=False, stop=True,
                    )
                    epilogue(m, pm)

        # ---- qkv GEMM -------------------------------------------------------
        # q,k channel-major: qkT[:, m, :], m in [0,16) covers channels [0,2C)
        qkT = resident.tile([128, MQK, S], bf16, name="qkT", tag="qkT")
        pt_qkv = pv_psum("pt_qkv")
        tT_qkv = None
        for g in range(4):            # groups of 4 output chunks
            pga = qk2_psum(f"pqk{g}a")
            pgb = qk2_psum(f"pqk{g}b")
            halves = (pga, pgb)
            for kc in range(KC):
                wt = wpool.tile([128, 512], bf16, tag="w")
                nc.sync.dma_start(
                    wt[:], qkv_w_r[kc, :, g * 512:(g + 1) * 512]
                )
                for i in range(4):
                    nc.tensor.matmul(
                        halves[i // 2][:, i % 2, :],
                        wt[:, i * 128:(i + 1) * 128],
                        xT[:, kc, :], start=(kc == 0), stop=False,
                    )
                if g == 0:
                    lora_step("qkv", pt_qkv, xT, kc)
            if g == 0:
                tT_qkv = lora_end("qkv", pt_qkv)
            for i in range(4):
                m = g * 4 + i
                nc.tensor.matmul(
                    halves[i // 2][:, i % 2, :],
                    lb["qkv"][:, m * 128:(m + 1) * 128],
                    tT_qkv[:], start=False, stop=True,
                )
            nc.any.tensor_copy(qkT[:, g * 4:g * 4 + 2, :], pga[:])
            nc.any.tensor_copy(qkT[:, g * 4 + 2:g * 4 + 4, :], pgb[:])

        # v token-major with interleaved ones columns: v[:, c, h*65:+64];
        # masked key rows (incl. their ones entries) are zeroed -> the mask
        # needs no separate handling anywhere else.
        v = resident.tile([128, 4, H * VSTRIDE], bf16, name="vtok", tag="vtok")
        for h in range(H):
            nc.vector.memset(
                v[:, :, h * VSTRIDE + HD:h * VSTRIDE + HD + 1], 1.0
            )
        for c in range(4):
            ones_cols = v[:, c, :].rearrange("p (h z) -> p h z", z=VSTRIDE)[
                :, :, HD:HD + 1
            ]
            nc.vector.tensor_scalar_mul(ones_cols, ones_cols, mask01[:, c:c + 1])
        for n in range(2):
            pga = qk2_psum(f"pv{n}a")
            pgb = qk2_psum(f"pv{n}b")
            halves = (pga, pgb)
            for kc in range(KC):
                wt = wpool.tile([128, 512], bf16, tag="w")
                nc.sync.dma_start(
                    wt[:], qkv_w_r[kc, :, 2 * C + n * 512:2 * C + (n + 1) * 512]
                )
                for c in range(4):
                    nc.tensor.matmul(
                        halves[c // 2][:, c % 2, :],
                        xT[:, kc, c * 128:(c + 1) * 128],
                        wt[:], start=(kc == 0), stop=False,
                    )
            for c in range(4):
                pm = halves[c // 2][:, c % 2, :]
                nc.tensor.matmul(
                    pm, tT_qkv[:, c * 128:(c + 1) * 128],
                    lb["qkv"][:, 2 * C + n * 512:2 * C + (n + 1) * 512],
                    start=False, stop=True,
                )
                # copy 8 heads' columns into 65-strided slots, zeroing masked
                # key rows on the way
                dst = v[:, c, n * 8 * VSTRIDE:(n + 1) * 8 * VSTRIDE].rearrange(
                    "p (h z) -> p h z", z=VSTRIDE
                )[:, :, 0:HD]
                src = pm.rearrange("p (h z) -> p h z", z=HD)
                nc.vector.tensor_scalar_mul(dst, src, mask01[:, c:c + 1])

        # ---- attention ------------------------------------------------------
        # xou: unnormalized attention output, channel-major [128, KC, S]
        xou = resident.tile([128, KC, S], bf16, name="xou", tag="xou")
        den128 = resident.tile([128, H, 4], f32r, name="den128", tag="den128")
        recip128 = resident.tile(
            [128, H, 4], f32r, name="recip128", tag="recip128"
        )
        recip8 = [
            resident.tile([8, S], f32r, name=f"recip8_{hb}", tag=f"recip8_{hb}")
            for hb in range(2)
        ]

        def finish_head(ph, ppv):
            pj, phalf = ph // 2, ph % 2
            tmd = tmpp.tile([128, S], f32r, name="tmd", tag="tmpd")
            nc.vector.tensor_copy(tmd[HD:HD + 1, :], ppv[HD:HD + 1, :])
            nc.sync.dma_start(den128[:, ph, :], tmd[HD:HD + 1, :])
            with nc.allow_low_precision(reason="f32r keeps fp32 bits"):
                nc.vector.reciprocal(recip128[:, ph, :], den128[:, ph, :])
            if phalf == 0:
                nc.vector.tensor_copy(xou[0:64, pj, :], ppv[0:HD, :])
            else:
                tmb = tmpp.tile([128, S], bf16, name="tmb", tag="tmpb")
                nc.vector.tensor_copy(tmb[0:HD, :], ppv[0:HD, :])
                nc.sync.dma_start(xou[64:128, pj, :], tmb[0:HD, :])

        def norm_half(hb):
            # heads [hb*8, hb*8+8): move their reciprocals to queries-on-free
            # layout via PE transposes, broadcast per chunk with a K=16
            # selection matmul, and scale xou chunks [hb*4, hb*4+4).  Runs
            # mid-attention for the first half so fc1's early chunks unblock.
            for cq in range(4):
                tp = qk2_psum(f"tp{hb}{cq}", dt=f32r)
                nc.tensor.transpose(
                    tp[0:8, 0, 0:128], recip128[:, hb * 8:hb * 8 + 8, cq],
                    ident[:],
                )
                nc.vector.tensor_copy(
                    recip8[hb][:, :].rearrange("h (p c) -> h p c", c=4)[
                        :, :, cq
                    ],
                    tp[0:8, 0, 0:128],
                )
            for jj in range(4):
                j = hb * 4 + jj
                pn = qk2_psum(f"pn{j}")
                nc.tensor.matmul(
                    pn[:, 0, :], sel8[:, jj * 128:(jj + 1) * 128],
                    recip8[hb][:],
                )
                nc.vector.tensor_mul(xou[:, j, :], xou[:, j, :], pn[:, 0, :])

        prev = None
        for h in range(H):
            j, half = h // 2, h % 2
            p0 = 64 * half
            qkA = qk2_psum("qkA")
            qkB = qk2_psum("qkB")
            pvt = pv_psum("pvt")
            exp_t = expp.tile([128, 4, S], bf16, name="exp_t", tag="exp")
            # interleave this head's QK with the previous head's PV 1:1: the
            # PE runs its stream in order, so PV (which waits on exp) must
            # not precede the next head's QK; alternating targets also avoids
            # same-bank accumulation stalls.
            for c in range(4):
                qk_dst = qkA[:, c, :] if c < 2 else qkB[:, c - 2, :]
                nc.tensor.matmul(
                    qk_dst,
                    qkT[p0:p0 + 64, 8 + j, c * 128:(c + 1) * 128],
                    qkT[p0:p0 + 64, j, :],
                )
                if prev is not None:
                    ph, pexp, ppv = prev
                    nc.tensor.matmul(
                        ppv[0:VSTRIDE, :],
                        v[:, c, ph * VSTRIDE:(ph + 1) * VSTRIDE],
                        pexp[:, c, :],
                        start=(c == 0), stop=(c == 3),
                    )
                if c == 1:
                    nc.scalar.activation(
                        exp_t[:, 0:2, :], qkA[:], AF.Exp, scale=0.125
                    )
                elif c == 3:
                    nc.scalar.activation(
                        exp_t[:, 2:4, :], qkB[:], AF.Exp, scale=0.125
                    )
            if prev is not None:
                finish_head(prev[0], prev[2])
            prev = (h, exp_t, pvt)
        ph, pexp, ppv = prev
        for c in range(4):
            nc.tensor.matmul(
                ppv[0:VSTRIDE, :],
                v[:, c, ph * VSTRIDE:(ph + 1) * VSTRIDE],
                pexp[:, c, :],
                start=(c == 0), stop=(c == 3),
            )
        finish_head(ph, ppv)
        norm_half(0)
        norm_half(1)
        xoT = xou  # normalized in place

        # ---- MLP fc1 + gelu -------------------------------------------------
        gT = resident.tile([128, KC, S], bf16, name="gT", tag="gT")

        def fc1_epi(m, pm):
            nc.scalar.activation(
                gT[:, m, :], pm, AF.Gelu, bias=biases["fc1"][:, m:m + 1]
            )

        mlp_gemm("fc1", fc1_w_r, xoT, fc1_epi)

        # ---- MLP fc2 + residual --------------------------------------------
        xo2T = resident.tile([128, KC, S], bf16, name="xo2T", tag="xo2T")

        def fc2_epi(m, pm):
            # xo2 = (fc2_psum + bias) + xo  (residual)
            nc.vector.scalar_tensor_tensor(
                xo2T[:, m, :], pm, biases["fc2"][:, m:m + 1],
                xoT[:, m, :], op0=ALU.add, op1=ALU.add,
            )

        mlp_gemm("fc2", fc2_w_r, gT, fc2_epi)

        # ---- proj -----------------------------------------------------------
        outT_r = outT_d[:].rearrange("(m p) s -> p m s", p=128)
        ots = {}

        def proj_epi(m, pm):
            ot = outp.tile([128, S], f32, name=f"ot{m}", tag="out")
            nc.scalar.activation(
                ot[:], pm, AF.Identity, bias=biases["proj"][:, m:m + 1]
            )
            nc.sync.dma_start(outT_r[:, m, :], ot[:])

        mlp_gemm("proj", proj_w_r, xo2T, proj_epi)

    nc.compile()
    _cache["nc"] = nc
    return nc


def _bf16(a):
    import ml_dtypes

    return np.asarray(a, dtype=np.float32).astype(ml_dtypes.bfloat16)


def _make_in_maps(inputs):
    x = np.asarray(inputs["x"], dtype=np.float32)
    mask = np.asarray(inputs["mask"])
    sel8 = np.zeros((8, 512), dtype=np.float32)
    for jj in range(4):
        for p in range(128):
            sel8[2 * jj + p // 64, jj * 128 + p] = 1.0
    shared = {"sel8": sel8, "ident": np.eye(128, dtype=np.float32)}
    for k in (
        "qkv_w", "qkv_la", "qkv_lb", "proj_w", "proj_la", "proj_lb",
        "fc1_w", "fc1_la", "fc1_lb", "fc2_w", "fc2_la", "fc2_lb",
    ):
        shared[k] = np.ascontiguousarray(_bf16(inputs[k]))
    for k in ("proj_b", "fc1_b", "fc2_b"):
        shared[k] = np.ascontiguousarray(inputs[k], dtype=np.float32)
    in_maps = []
    for b in range(NCORES):
        m01 = mask[b, :S].astype(np.float32)          # 1.0 keep / 0.0 drop
        in_maps.append(
            dict(
                shared,
                xT=np.ascontiguousarray(_bf16(x[b].T)),
                mask01=np.ascontiguousarray(m01.reshape(4, 128).T),
            )
        )
    return in_maps


def _run(inputs, trace=False):
    from concourse.bass_utils import run_bass_kernel_spmd

    nc = _get_nc()
    in_maps = _make_in_maps(inputs)
    res = run_bass_kernel_spmd(nc, in_maps, list(range(NCORES)), trace=trace)
    out = np.stack(
        [np.ascontiguousarray(res.results[b]["outT"].T) for b in range(NCORES)]
    )
    return out, res


def kernel(**inputs):
    out, _ = _run(inputs, trace=False)
    return out

